# revision 8
# baseline (speedup 1.0000x reference)
"""GAT (2-layer, PyG-style) on 8 Trainium2 NeuronCores.

Strategy (per sharding hint): nodes partitioned across 8 cores by dst range;
edge list dst-sorted and sharded by destination so segment-softmax and
scatter-add stay core-local. Three SPMD launches:
  L1 "node":  per-core node shard: h1 = x@W1, alpha_src/dst  -> gather-table rows
  L2 "edge1": per-core dst shard:  layer-1 attention aggregation -> x2 rows
  L3 "edge2": layer-2 aggregation with postponed W2 matmul -> output shard
Host does only layout between launches (byte-assembly of gather tables).

Edge phase per 128-edge tile: dma_gather of 512B table rows by src
(fp16 features + fp32 meta packed in fp32-typed rows; the signed-int16
index limit is handled by lo/hi table halves, with all lo tiles ordered
before all hi tiles so gathers run in large batches at full descriptor
rate). A one-hot matrix M01 (DVE compare vs constant iota) turns the
per-dst-block scatter-add + softmax denominators into one TensorE matmul
per tile; PSUM group-partials are added into a per-block SBUF accumulator
strip. alpha_dst is expanded edge-wise with a per-tile transpose(M01)
matmul. Softmax skips the segment-max subtraction (exact rescaling
invariance; |e| <~ 8 so exp() is safe in fp32).
"""

import math
import numpy as np

import concourse.bass as bass
import concourse.bacc as bacc
import concourse.mybir as mybir
import concourse.tile as tile
from concourse.bass_utils import run_bass_kernel_spmd

P = 128
NEG_SLOPE = 0.2
N_CORES = 8
GMAX = 48           # tiles per dma_gather group
LO_ROWS = 32768     # int16 index limit for dma_gather
F16_ONES_F32 = np.frombuffer(np.array([15360, 15360], dtype=np.uint16).tobytes(),
                             dtype=np.float32)[0]  # two fp16 1.0s as one f32

# table row layout, in f32 slots (512B rows)
ROW_SLOTS = 128
SL_FEAT = 0      # 64 slots: 128 fp16 features
SL_ONES = 64     # 2 slots: 4 fp16 ones (rhs denominator columns)
SL_AS = 66       # 4 slots: alpha_src (f32, per head; layer2 uses slot 66 only)
SL_AD = 70       # 4 slots: alpha_dst (f32)

dt = mybir.dt

EXECUTOR = None  # test hook: callable(nc, in_maps) -> list[dict]; None = hardware


def _execute(nc, in_maps):
    if EXECUTOR is not None:
        return EXECUTOR(nc, in_maps)
    return run_bass_kernel_spmd(nc, in_maps, list(range(len(in_maps)))).results


# ----------------------------------------------------------------------------
# host-side preprocessing
# ----------------------------------------------------------------------------

def _prep_edges(src, dst, n, n_cores):
    """dst-sort, shard by dst range, tile into 128-edge tiles.

    Tile order: all lo-side tiles (src < LO_ROWS) of all blocks, then all
    hi-side tiles — so dma_gather batches are as large as possible. Per-block
    tile counts are uniform across cores (padded with dead edges)."""
    nd = n // n_cores
    nb = math.ceil(nd / P)
    order = np.argsort(dst, kind="stable")
    src, dst = src[order], dst[order]
    starts = np.searchsorted(dst, np.arange(0, n + 1))

    side_edges = [[[None, None] for _ in range(nb)] for _ in range(n_cores)]
    for c in range(n_cores):
        base = c * nd
        for b in range(nb):
            lo_d = base + b * P
            hi_d = min(base + (b + 1) * P, base + nd)
            e0, e1 = starts[lo_d], starts[hi_d]
            s = src[e0:e1]
            d = dst[e0:e1]
            m = s < LO_ROWS
            side_edges[c][b][0] = (s[m], d[m] - lo_d)
            side_edges[c][b][1] = (s[~m] - LO_ROWS, d[~m] - lo_d)

    nt_side = np.zeros((nb, 2), dtype=int)
    for b in range(nb):
        for sd in range(2):
            mx = max(len(side_edges[c][b][sd][0]) for c in range(n_cores))
            nt_side[b, sd] = math.ceil(mx / P) if mx else 0
        if nt_side[b].sum() == 0:
            nt_side[b, 0] = 1  # keep at least one tile so the acc gets written
    ntt = int(nt_side.sum())

    # tile list: lo tiles of all blocks, then hi tiles of all blocks
    tiles = []
    for sd in range(2):
        for b in range(nb):
            tiles += [(b, sd)] * nt_side[b, sd]

    idx16 = np.zeros((n_cores, 128, ntt * P // 16), dtype=np.int16)
    dstl = np.full((n_cores, 128, ntt), 999.0, dtype=np.float16)
    off_bs = {}
    off = 0
    for sd in range(2):
        for b in range(nb):
            off_bs[(b, sd)] = off
            off += nt_side[b, sd]
    for c in range(n_cores):
        for b in range(nb):
            for sd in range(2):
                s, dl = side_edges[c][b][sd]
                ntil = nt_side[b, sd]
                if ntil == 0:
                    continue
                o = off_bs[(b, sd)]
                sp = np.zeros(ntil * P, dtype=np.int16)
                dp = np.full(ntil * P, 999.0, dtype=np.float16)
                sp[:len(s)] = s.astype(np.int16)
                dp[:len(s)] = dl.astype(np.float16)
                i = np.arange(ntil * P)
                dstl[c, i % P, o + i // P] = dp
                for j in range(ntil):
                    seg = sp[j * P:(j + 1) * P]
                    g = np.zeros((16, 8), dtype=np.int16)
                    g[np.arange(P) % 16, np.arange(P) // 16] = seg
                    idx16[c, :, (o + j) * 8:(o + j + 1) * 8] = np.tile(g, (8, 1))

    return {
        "nd": nd, "nb": nb, "ntt": ntt, "tiles": tiles,
        "nt_side": nt_side, "idx16": idx16, "dstl": dstl,
    }


def _gather_groups(meta):
    """Groups of consecutive same-side tiles (may span blocks), up to GMAX."""
    groups = []  # (tile_start, ntiles, side)
    tiles = meta["tiles"]
    j = 0
    while j < len(tiles):
        sd = tiles[j][1]
        k = j
        while k < len(tiles) and tiles[k][1] == sd and k - j < GMAX:
            k += 1
        groups.append((j, k - j, sd))
        j = k
    return groups


# ----------------------------------------------------------------------------
# device kernels
# ----------------------------------------------------------------------------

def build_node_kernel(nd_pad, reps=1):
    """L1: x_shard [nd_pad,128] f32 -> nodeout [nd_pad,72] f32
    (= [h fp16 x128 | as f32 x4 | ad f32 x4])."""
    nc = bacc.Bacc("TRN2", target_bir_lowering=False, debug=False)
    x = nc.dram_tensor("x", [nd_pad, P], dt.float32, kind="ExternalInput").ap()
    w1 = nc.dram_tensor("w1", [P, P], dt.float32, kind="ExternalInput").ap()
    amat = nc.dram_tensor("amat", [P, 8], dt.float32, kind="ExternalInput").ap()
    nout = nc.dram_tensor("nodeout", [nd_pad, 72], dt.float32,
                          kind="ExternalOutput").ap()
    nt = nd_pad // P

    with tile.TileContext(nc) as tc:
        with tc.tile_pool(name="const", bufs=1) as cpool, \
             tc.tile_pool(name="sbuf", bufs=3) as pool, \
             tc.tile_pool(name="psum", bufs=1, space="PSUM") as pp:
            from concourse.masks import make_identity
            ident = cpool.tile([P, P], dt.float16)
            make_identity(nc, ident[:])
            w1f = cpool.tile([P, P], dt.float32)
            nc.sync.dma_start(out=w1f[:], in_=w1[:])
            w116 = cpool.tile([P, P], dt.float16)
            nc.vector.tensor_copy(out=w116[:], in_=w1f[:])
            amf = cpool.tile([P, 8], dt.float32)
            nc.sync.dma_start(out=amf[:], in_=amat[:])
            am16 = cpool.tile([P, 8], dt.float16)
            nc.vector.tensor_copy(out=am16[:], in_=amf[:])

            def node_body():
              for t in range(nt):
                rows = slice(t * P, (t + 1) * P)
                xt = pool.tile([P, P], dt.float32, tag="xt")
                nc.sync.dma_start(out=xt[:], in_=x[rows, :])
                x16 = pool.tile([P, P], dt.float16, tag="x16")
                nc.vector.tensor_copy(out=x16[:], in_=xt[:])
                ps_xt = pp.tile([P, P], dt.float16, tag="tp")
                nc.tensor.transpose(out=ps_xt[:], in_=x16[:], identity=ident[:])
                xT = pool.tile([P, P], dt.float16, tag="xT")
                nc.scalar.copy(out=xT[:], in_=ps_xt[:])
                ot = pool.tile([P, 72], dt.float32, tag="ot")
                # h = xT.T @ W1   [node, f] -> fp16 into slots 0:64
                ps_h = pp.tile([P, P], dt.float32, tag="h")
                nc.tensor.matmul(out=ps_h[:], lhsT=xT[:], rhs=w116[:],
                                 start=True, stop=True)
                nc.scalar.copy(out=ot[:, 0:64].bitcast(dt.float16), in_=ps_h[:])
                # hT = W1.T @ xT  [f, node]
                ps_hT = pp.tile([P, P], dt.float32, tag="hT")
                nc.tensor.matmul(out=ps_hT[:], lhsT=w116[:], rhs=xT[:],
                                 start=True, stop=True)
                hT16 = pool.tile([P, P], dt.float16, tag="hT16")
                nc.scalar.copy(out=hT16[:], in_=ps_hT[:])
                # asadT = amat.T @ hT  [8, node]
                ps_a = pp.tile([8, P], dt.float32, tag="aT")
                nc.tensor.matmul(out=ps_a[:], lhsT=am16[:], rhs=hT16[:],
                                 start=True, stop=True)
                aT16 = pool.tile([8, P], dt.float16, tag="aT16")
                nc.scalar.copy(out=aT16[:], in_=ps_a[:])
                ps_at = pp.tile([P, 8], dt.float16, tag="a")
                nc.tensor.transpose(out=ps_at[:], in_=aT16[:],
                                    identity=ident[:8, :8])
                nc.vector.tensor_copy(out=ot[:, 64:72], in_=ps_at[:])
                nc.sync.dma_start(out=nout[rows, :], in_=ot[:])

            if reps == 1:
                node_body()
            else:
                with tc.For_i(0, reps, 1):
                    node_body()
    nc.compile()
    return nc


def build_edge_kernel(meta, layer, n, nd_pad, b_nonzero, reps=1):
    """L2/L3: one attention layer over the core's dst shard.

    layer==1: H=4 heads; epilogue -> [x2 fp16 x128 | as2 | ad2] rows (66 f32).
    layer==2: H=1; epilogue -> @W2 + bias -> out f32 [nd_pad, 128].
    """
    H = 4 if layer == 1 else 1
    RW = P + H
    ntt, nb = meta["ntt"], meta["nb"]
    tiles = meta["tiles"]
    groups = _gather_groups(meta)

    nc = bacc.Bacc("TRN2", target_bir_lowering=False, debug=False)
    table = nc.dram_tensor("table", [n, ROW_SLOTS], dt.float32,
                           kind="ExternalInput").ap()
    idx_d = nc.dram_tensor("idx16", [128, ntt * 8], dt.int16,
                           kind="ExternalInput").ap()
    dstl_d = nc.dram_tensor("dstl", [128, ntt], dt.float16,
                            kind="ExternalInput").ap()
    adrow_d = nc.dram_tensor("adrow", [nd_pad, 4], dt.float32,
                             kind="ExternalInput").ap()
    if layer == 1:
        ws2_d = nc.dram_tensor("ws2", [1, 2 * P], dt.float32,
                               kind="ExternalInput").ap()
        b1_d = nc.dram_tensor("b1", [1, P], dt.float32, kind="ExternalInput").ap()
        x2o = nc.dram_tensor("x2m", [nd_pad, 66], dt.float32,
                             kind="ExternalOutput").ap()
    else:
        w2_d = nc.dram_tensor("w2", [P, P], dt.float32, kind="ExternalInput").ap()
        b2_d = nc.dram_tensor("b2", [1, P], dt.float32, kind="ExternalInput").ap()
        outo = nc.dram_tensor("out", [nd_pad, P], dt.float32,
                              kind="ExternalOutput").ap()

    with tile.TileContext(nc) as tc:
        with tc.tile_pool(name="const", bufs=1) as cpool, \
             tc.tile_pool(name="res", bufs=1) as rpool, \
             tc.tile_pool(name="g", bufs=2) as gpool, \
             tc.tile_pool(name="w", bufs=2) as wpool, \
             tc.tile_pool(name="bl", bufs=2) as bpool, \
             tc.tile_pool(name="psum", bufs=2, space="PSUM") as pp:
            from concourse.masks import make_identity
            ident = cpool.tile([P, P], dt.float16)
            make_identity(nc, ident[:])
            iota_i = cpool.tile([P, P], dt.int16)
            nc.gpsimd.iota(iota_i[:], pattern=[[1, P]], base=0,
                           channel_multiplier=0)
            iota16 = cpool.tile([P, P], dt.float16)
            nc.vector.tensor_copy(out=iota16[:], in_=iota_i[:])

            if layer == 1:
                ws2 = cpool.tile([P, 2 * P], dt.float32)
                nc.sync.dma_start(
                    out=ws2[:], in_=ws2_d[0:1, :].to_broadcast([P, 2 * P]))
                b1t = cpool.tile([P, P], dt.float32)
                nc.sync.dma_start(out=b1t[:],
                                  in_=b1_d[0:1, :].to_broadcast([P, P]))
            else:
                w2f = cpool.tile([P, P], dt.float32)
                nc.sync.dma_start(out=w2f[:], in_=w2_d[:])
                w216 = cpool.tile([P, P], dt.float16)
                nc.vector.tensor_copy(out=w216[:], in_=w2f[:])
                b2t = cpool.tile([P, P], dt.float32)
                nc.sync.dma_start(out=b2t[:], in_=b2_d[0:1, :].to_broadcast([P, P]))

            idx_sb = rpool.tile([128, ntt * 8], dt.int16)
            nc.sync.dma_start(out=idx_sb[:], in_=idx_d[:])
            dstl_sb = rpool.tile([128, ntt], dt.float16)
            nc.sync.dma_start(out=dstl_sb[:], in_=dstl_d[:])
            # all blocks' ad columns resident: [p, b, h]
            adall = rpool.tile([128, nb * H], dt.float16)
            adall_f = rpool.tile([128, nb * 4], dt.float32)
            nc.sync.dma_start(
                out=adall_f[:].rearrange("p (b h) -> p b h", h=4),
                in_=adrow_d[:].rearrange("(b p) h -> p b h", p=P))
            nc.vector.tensor_copy(
                out=adall[:].rearrange("p (b h) -> p b h", h=H),
                in_=adall_f[:].rearrange("p (b h) -> p b h", h=4)[:, :, 0:H])

            # per-block accumulator strip in SBUF
            accs = rpool.tile([128, nb * RW], dt.float32)

            lo_view = table[0:LO_ROWS, :]
            hi_view = table[LO_ROWS:n, :]

            def layer_body():
                nc.vector.memset(accs[:], 0.0)
                for (gt0, gn, sd) in groups:
                    src_view = lo_view if sd == 0 else hi_view
                    gbuf = gpool.tile([128, GMAX * ROW_SLOTS], dt.float32,
                                      tag="gb")
                    nc.gpsimd.dma_gather(
                        out_ap=gbuf[:, :gn * ROW_SLOTS].rearrange(
                            "p (n e) -> p n e", e=ROW_SLOTS),
                        in_ap=src_view,
                        idxs_ap=idx_sb[:, gt0 * 8:(gt0 + gn) * 8],
                        num_idxs=gn * P,
                        num_idxs_reg=gn * P,
                        elem_size=ROW_SLOTS,
                        single_packet=False,
                    )
                    g16 = gbuf[:, :gn * ROW_SLOTS].bitcast(dt.float16)

                    m01 = wpool.tile([128, GMAX * P], dt.float16, tag="m01")
                    nc.vector.tensor_tensor(
                        out=m01[:, :gn * P].rearrange("p (n e) -> p n e", e=P),
                        in0=iota16[:].unsqueeze(1).to_broadcast([P, gn, P]),
                        in1=dstl_sb[:, gt0:gt0 + gn].unsqueeze(2).to_broadcast(
                            [P, gn, P]),
                        op=mybir.AluOpType.is_equal)

                    adx = pp.tile([P, GMAX * H], dt.float32, tag="adx")
                    for j in range(gn):
                        blk = tiles[gt0 + j][0]
                        ps_t = pp.tile([P, P], dt.float16, tag="tp")
                        nc.tensor.transpose(
                            out=ps_t[:], in_=m01[:, j * P:(j + 1) * P],
                            identity=ident[:])
                        m01T = wpool.tile([P, P], dt.float16, tag="m01T")
                        nc.scalar.copy(out=m01T[:], in_=ps_t[:])
                        nc.tensor.matmul(
                            out=adx[:, j * H:(j + 1) * H], lhsT=m01T[:],
                            rhs=adall[:, blk * H:(blk + 1) * H],
                            start=True, stop=True)

                    e_s = wpool.tile([128, GMAX * H], dt.float32, tag="es")
                    nc.vector.tensor_tensor(
                        out=e_s[:, :gn * H].rearrange("p (n e) -> p n e", e=H),
                        in0=gbuf[:, :gn * ROW_SLOTS].rearrange(
                            "p (n e) -> p n e", e=ROW_SLOTS)[:, :, SL_AS:SL_AS + H],
                        in1=adx[:, :gn * H].rearrange("p (n e) -> p n e", e=H),
                        op=mybir.AluOpType.add)
                    e_m = wpool.tile([128, GMAX * H], dt.float32, tag="em")
                    nc.vector.tensor_scalar_mul(
                        out=e_m[:, :gn * H], in0=e_s[:, :gn * H],
                        scalar1=NEG_SLOPE)
                    nc.vector.tensor_tensor(
                        out=e_s[:, :gn * H], in0=e_s[:, :gn * H],
                        in1=e_m[:, :gn * H], op=mybir.AluOpType.max)
                    ex16 = wpool.tile([128, GMAX * H], dt.float16, tag="ex")
                    nc.scalar.activation(ex16[:, :gn * H], e_s[:, :gn * H],
                                         mybir.ActivationFunctionType.Exp)

                    rhs = wpool.tile([128, GMAX * RW], dt.float16, tag="rhs")
                    nc.vector.tensor_tensor(
                        out=rhs[:, :gn * RW].rearrange(
                            "p (n e) -> p n e", e=RW)[:, :, 0:P].rearrange(
                            "p n (h c) -> p n h c", h=H),
                        in0=g16.rearrange("p (n e) -> p n e", e=2 * ROW_SLOTS)[
                            :, :, 0:P].rearrange("p n (h c) -> p n h c", h=H),
                        in1=ex16[:, :gn * H].rearrange(
                            "p (n h) -> p n h", h=H).unsqueeze(3).to_broadcast(
                            [128, gn, H, P // H]),
                        op=mybir.AluOpType.mult)
                    nc.vector.tensor_copy(
                        out=rhs[:, :gn * RW].rearrange(
                            "p (n e) -> p n e", e=RW)[:, :, P:RW],
                        in_=ex16[:, :gn * H].rearrange("p (n h) -> p n h", h=H))

                    # main scatter matmuls: per contiguous block piece
                    j = 0
                    while j < gn:
                        blk = tiles[gt0 + j][0]
                        k = j
                        while k < gn and tiles[gt0 + k][0] == blk:
                            k += 1
                        pacc = pp.tile([P, RW], dt.float32, tag="acc")
                        for q in range(j, k):
                            nc.tensor.matmul(
                                out=pacc[:],
                                lhsT=m01[:, q * P:(q + 1) * P],
                                rhs=rhs[:, q * RW:(q + 1) * RW],
                                start=(q == j), stop=(q == k - 1))
                        nc.vector.tensor_tensor(
                            out=accs[:, blk * RW:(blk + 1) * RW],
                            in0=accs[:, blk * RW:(blk + 1) * RW],
                            in1=pacc[:], op=mybir.AluOpType.add)
                        j = k

                # ---- epilogue over all blocks ----
                for b in range(nb):
                    acc = accs[:, b * RW:(b + 1) * RW]
                    rec = bpool.tile([P, H], dt.float32, tag="rec")
                    nc.vector.reciprocal(out=rec[:], in_=acc[:, P:RW])
                    xr = bpool.tile([P, P], dt.float32, tag="xr")
                    nc.vector.tensor_tensor(
                        out=xr[:].rearrange("p (h c) -> p h c", h=H),
                        in0=acc[:, 0:P].rearrange("p (h c) -> p h c", h=H),
                        in1=rec[:].unsqueeze(2).to_broadcast([P, H, P // H]),
                        op=mybir.AluOpType.mult)
                    rows = slice(b * P, (b + 1) * P)
                    if layer == 1:
                        if b_nonzero:
                            nc.vector.tensor_tensor(
                                out=xr[:], in0=xr[:], in1=b1t[:],
                                op=mybir.AluOpType.add)
                        nc.vector.tensor_scalar_max(out=xr[:], in0=xr[:],
                                                    scalar1=0.0)
                        om = bpool.tile([P, 66], dt.float32, tag="om")
                        nc.vector.tensor_copy(out=om[:, 0:64].bitcast(dt.float16),
                                              in_=xr[:])
                        t2 = bpool.tile([P, 2 * P], dt.float32, tag="t2")
                        nc.vector.tensor_tensor(
                            out=t2[:].rearrange("p (a c) -> p a c", a=2),
                            in0=xr[:].unsqueeze(1).to_broadcast([P, 2, P]),
                            in1=ws2[:].rearrange("p (a c) -> p a c", a=2),
                            op=mybir.AluOpType.mult)
                        nc.vector.tensor_reduce(
                            out=om[:, 64:66],
                            in_=t2[:].rearrange("p (a c) -> p a c", a=2),
                            axis=mybir.AxisListType.X, op=mybir.AluOpType.add)
                        nc.sync.dma_start(out=x2o[rows, :], in_=om[:])
                    else:
                        x16b = bpool.tile([P, P], dt.float16, tag="xab")
                        nc.vector.tensor_copy(out=x16b[:], in_=xr[:])
                        ps_t2 = pp.tile([P, P], dt.float16, tag="tp")
                        nc.tensor.transpose(out=ps_t2[:], in_=x16b[:],
                                            identity=ident[:])
                        aggT = bpool.tile([P, P], dt.float16, tag="aggT")
                        nc.scalar.copy(out=aggT[:], in_=ps_t2[:])
                        ps_o = pp.tile([P, P], dt.float32, tag="acc")
                        nc.tensor.matmul(out=ps_o[:], lhsT=aggT[:], rhs=w216[:],
                                         start=True, stop=True)
                        ot = bpool.tile([P, P], dt.float32, tag="ot")
                        if b_nonzero:
                            nc.vector.tensor_tensor(
                                out=ot[:], in0=ps_o[:], in1=b2t[:],
                                op=mybir.AluOpType.add)
                        else:
                            nc.vector.tensor_copy(out=ot[:], in_=ps_o[:])
                        nc.sync.dma_start(out=outo[rows, :], in_=ot[:])

            if reps == 1:
                layer_body()
            else:
                with tc.For_i(0, reps, 1):
                    layer_body()
    nc.compile()
    return nc


# ----------------------------------------------------------------------------
# host orchestration
# ----------------------------------------------------------------------------

def _assemble_table(n, feat16, asv, adv):
    t = np.zeros((n, ROW_SLOTS), dtype=np.float32)
    t[:, SL_FEAT:SL_FEAT + 64] = np.ascontiguousarray(feat16).view(np.float32)
    t[:, SL_ONES:SL_ONES + 2] = F16_ONES_F32
    t[:, SL_AS:SL_AS + asv.shape[1]] = asv
    t[:, SL_AD:SL_AD + adv.shape[1]] = adv
    return t


def kernel(x, edge_index, W1, att_src1, att_dst1, b1, W2, att_src2, att_dst2, b2):
    x = np.asarray(x, np.float32)
    n = x.shape[0]
    ei = np.asarray(edge_index).astype(np.int64)
    loops = np.arange(n, dtype=np.int64)
    src = np.concatenate([ei[0], loops])
    dst = np.concatenate([ei[1], loops])
    W1 = np.asarray(W1, np.float32)
    W2 = np.asarray(W2, np.float32)
    a_s1 = np.asarray(att_src1, np.float32).reshape(4, 32)
    a_d1 = np.asarray(att_dst1, np.float32).reshape(4, 32)
    b1 = np.asarray(b1, np.float32).reshape(-1)
    b2 = np.asarray(b2, np.float32).reshape(-1)
    a_s2 = np.asarray(att_src2, np.float32).reshape(-1)
    a_d2 = np.asarray(att_dst2, np.float32).reshape(-1)

    meta = _prep_edges(src, dst, n, N_CORES)
    nd, nb = meta["nd"], meta["nb"]
    nd_pad = nb * P

    # L1: node kernel (sharded by node)
    nc1 = build_node_kernel(nd_pad)
    amat = np.zeros((P, 8), dtype=np.float32)
    for h in range(4):
        amat[h * 32:(h + 1) * 32, h] = a_s1[h]
        amat[h * 32:(h + 1) * 32, 4 + h] = a_d1[h]
    in1 = []
    for c in range(N_CORES):
        xs = np.zeros((nd_pad, P), np.float32)
        xs[:nd] = x[c * nd:(c + 1) * nd]
        in1.append({"x": xs, "w1": W1, "amat": amat})
    r1 = _execute(nc1, in1)

    nodeout = np.concatenate([r1[c]["nodeout"][:nd] for c in range(N_CORES)])
    h16 = np.ascontiguousarray(nodeout[:, 0:64]).view(np.float16)
    asad1 = nodeout[:, 64:72]
    table1 = _assemble_table(n, h16, asad1[:, 0:4], asad1[:, 4:8])

    # L2: edge layer 1
    ws2 = np.concatenate([W2 @ a_s2, W2 @ a_d2]).astype(np.float32).reshape(1, -1)
    nc2 = build_edge_kernel(meta, 1, n, nd_pad, b_nonzero=bool(np.any(b1)))
    in2 = []
    for c in range(N_CORES):
        adrow = np.zeros((nd_pad, 4), np.float32)
        adrow[:nd] = asad1[c * nd:(c + 1) * nd, 4:8]
        in2.append({"table": table1, "idx16": meta["idx16"][c],
                    "dstl": meta["dstl"][c], "adrow": adrow, "ws2": ws2,
                    "b1": b1.reshape(1, -1)})
    r2 = _execute(nc2, in2)

    x2m = np.concatenate([r2[c]["x2m"][:nd] for c in range(N_CORES)])
    x2_16 = np.ascontiguousarray(x2m[:, 0:64]).view(np.float16)
    asad2 = x2m[:, 64:66]
    table2 = _assemble_table(n, x2_16, asad2[:, 0:1], asad2[:, 1:2])

    # L3: edge layer 2
    nc3 = build_edge_kernel(meta, 2, n, nd_pad, b_nonzero=bool(np.any(b2)))
    in3 = []
    for c in range(N_CORES):
        adrow = np.zeros((nd_pad, 4), np.float32)
        adrow[:nd, 0] = asad2[c * nd:(c + 1) * nd, 1]
        in3.append({"table": table2, "idx16": meta["idx16"][c],
                    "dstl": meta["dstl"][c], "adrow": adrow,
                    "w2": W2, "b2": b2.reshape(1, -1)})
    r3 = _execute(nc3, in3)

    out = np.concatenate([r3[c]["out"][:nd] for c in range(N_CORES)])
    return out.astype(np.float32)


# revision 9
# speedup vs baseline: 1.1867x; 1.1867x over previous
"""GAT (2-layer, PyG-style) on 8 Trainium2 NeuronCores.

Strategy (per sharding hint): nodes partitioned across 8 cores by dst range;
edge list dst-sorted and sharded by destination so segment-softmax and
scatter-add stay core-local. Three SPMD launches:
  L1 "node":  per-core node shard: h1 = x@W1, alpha_src/dst  -> gather-table rows
  L2 "edge1": per-core dst shard:  layer-1 attention aggregation -> x2 rows
  L3 "edge2": layer-2 aggregation with postponed W2 matmul -> output shard
Host does only layout between launches (byte-assembly of gather tables).

Edge phase per 128-edge tile: dma_gather of 512B table rows by src
(fp16 features + fp32 meta packed in fp32-typed rows; the signed-int16
index limit is handled by lo/hi table halves, with all lo tiles ordered
before all hi tiles so gathers run in large batches at full descriptor
rate). A one-hot matrix M01 (DVE compare vs constant iota) turns the
per-dst-block scatter-add + softmax denominators into one TensorE matmul
per tile; PSUM group-partials are added into a per-block SBUF accumulator
strip. alpha_dst is expanded edge-wise with a per-tile transpose(M01)
matmul. Softmax skips the segment-max subtraction (exact rescaling
invariance; |e| <~ 8 so exp() is safe in fp32).
"""

import math
import numpy as np

import concourse.bass as bass
import concourse.bacc as bacc
import concourse.mybir as mybir
import concourse.tile as tile
from concourse.bass_utils import run_bass_kernel_spmd

P = 128
NEG_SLOPE = 0.2
N_CORES = 8
GMAX = 32           # tiles per dma_gather group
LO_ROWS = 32768     # int16 index limit for dma_gather
F16_ONES_F32 = np.frombuffer(np.array([15360, 15360], dtype=np.uint16).tobytes(),
                             dtype=np.float32)[0]  # two fp16 1.0s as one f32

# table row layout, in f32 slots (512B rows)
ROW_SLOTS = 128
SL_FEAT = 0      # 64 slots: 128 fp16 features
SL_ONES = 64     # 2 slots: 4 fp16 ones (rhs denominator columns)
SL_AS = 66       # 4 slots: alpha_src (f32, per head; layer2 uses slot 66 only)
SL_AD = 70       # 4 slots: alpha_dst (f32)

dt = mybir.dt

EXECUTOR = None  # test hook: callable(nc, in_maps) -> list[dict]; None = hardware


def _execute(nc, in_maps):
    if EXECUTOR is not None:
        return EXECUTOR(nc, in_maps)
    return run_bass_kernel_spmd(nc, in_maps, list(range(len(in_maps)))).results


# ----------------------------------------------------------------------------
# host-side preprocessing
# ----------------------------------------------------------------------------

def _prep_edges(src, dst, n, n_cores):
    """dst-sort, shard by dst range, tile into 128-edge tiles.

    Tile order: all lo-side tiles (src < LO_ROWS) of all blocks, then all
    hi-side tiles — so dma_gather batches are as large as possible. Per-block
    tile counts are uniform across cores (padded with dead edges)."""
    nd = n // n_cores
    nb = math.ceil(nd / P)
    order = np.argsort(dst, kind="stable")
    src, dst = src[order], dst[order]
    starts = np.searchsorted(dst, np.arange(0, n + 1))

    side_edges = [[[None, None] for _ in range(nb)] for _ in range(n_cores)]
    for c in range(n_cores):
        base = c * nd
        for b in range(nb):
            lo_d = base + b * P
            hi_d = min(base + (b + 1) * P, base + nd)
            e0, e1 = starts[lo_d], starts[hi_d]
            s = src[e0:e1]
            d = dst[e0:e1]
            m = s < LO_ROWS
            side_edges[c][b][0] = (s[m], d[m] - lo_d)
            side_edges[c][b][1] = (s[~m] - LO_ROWS, d[~m] - lo_d)

    nt_side = np.zeros((nb, 2), dtype=int)
    for b in range(nb):
        for sd in range(2):
            mx = max(len(side_edges[c][b][sd][0]) for c in range(n_cores))
            nt_side[b, sd] = math.ceil(mx / P) if mx else 0
        if nt_side[b].sum() == 0:
            nt_side[b, 0] = 1  # keep at least one tile so the acc gets written
    ntt = int(nt_side.sum())

    # tile list: lo tiles of all blocks, then hi tiles of all blocks
    tiles = []
    for sd in range(2):
        for b in range(nb):
            tiles += [(b, sd)] * nt_side[b, sd]

    idx16 = np.zeros((n_cores, 128, ntt * P // 16), dtype=np.int16)
    dstl = np.full((n_cores, 128, ntt), 999.0, dtype=np.float16)
    off_bs = {}
    off = 0
    for sd in range(2):
        for b in range(nb):
            off_bs[(b, sd)] = off
            off += nt_side[b, sd]
    for c in range(n_cores):
        for b in range(nb):
            for sd in range(2):
                s, dl = side_edges[c][b][sd]
                ntil = nt_side[b, sd]
                if ntil == 0:
                    continue
                o = off_bs[(b, sd)]
                sp = np.zeros(ntil * P, dtype=np.int16)
                dp = np.full(ntil * P, 999.0, dtype=np.float16)
                sp[:len(s)] = s.astype(np.int16)
                dp[:len(s)] = dl.astype(np.float16)
                i = np.arange(ntil * P)
                dstl[c, i % P, o + i // P] = dp
                for j in range(ntil):
                    seg = sp[j * P:(j + 1) * P]
                    g = np.zeros((16, 8), dtype=np.int16)
                    g[np.arange(P) % 16, np.arange(P) // 16] = seg
                    idx16[c, :, (o + j) * 8:(o + j + 1) * 8] = np.tile(g, (8, 1))

    return {
        "nd": nd, "nb": nb, "ntt": ntt, "tiles": tiles,
        "nt_side": nt_side, "idx16": idx16, "dstl": dstl,
    }


def _gather_groups(meta):
    """Groups of consecutive same-side tiles (may span blocks), up to GMAX."""
    groups = []  # (tile_start, ntiles, side)
    tiles = meta["tiles"]
    j = 0
    while j < len(tiles):
        sd = tiles[j][1]
        k = j
        while k < len(tiles) and tiles[k][1] == sd and k - j < GMAX:
            k += 1
        groups.append((j, k - j, sd))
        j = k
    return groups


# ----------------------------------------------------------------------------
# device kernels
# ----------------------------------------------------------------------------

def build_node_kernel(nd_pad, reps=1):
    """L1: x_shard [nd_pad,128] f32 -> nodeout [nd_pad,72] f32
    (= [h fp16 x128 | as f32 x4 | ad f32 x4])."""
    nc = bacc.Bacc("TRN2", target_bir_lowering=False, debug=False)
    x = nc.dram_tensor("x", [nd_pad, P], dt.float32, kind="ExternalInput").ap()
    w1 = nc.dram_tensor("w1", [P, P], dt.float32, kind="ExternalInput").ap()
    amat = nc.dram_tensor("amat", [P, 8], dt.float32, kind="ExternalInput").ap()
    nout = nc.dram_tensor("nodeout", [nd_pad, 72], dt.float32,
                          kind="ExternalOutput").ap()
    nt = nd_pad // P

    with tile.TileContext(nc) as tc:
        with tc.tile_pool(name="const", bufs=1) as cpool, \
             tc.tile_pool(name="sbuf", bufs=3) as pool, \
             tc.tile_pool(name="psum", bufs=1, space="PSUM") as pp:
            from concourse.masks import make_identity
            ident = cpool.tile([P, P], dt.float16)
            make_identity(nc, ident[:])
            w1f = cpool.tile([P, P], dt.float32)
            nc.sync.dma_start(out=w1f[:], in_=w1[:])
            w116 = cpool.tile([P, P], dt.float16)
            nc.vector.tensor_copy(out=w116[:], in_=w1f[:])
            amf = cpool.tile([P, 8], dt.float32)
            nc.sync.dma_start(out=amf[:], in_=amat[:])
            am16 = cpool.tile([P, 8], dt.float16)
            nc.vector.tensor_copy(out=am16[:], in_=amf[:])

            def node_body():
              for t in range(nt):
                rows = slice(t * P, (t + 1) * P)
                xt = pool.tile([P, P], dt.float32, tag="xt")
                nc.sync.dma_start(out=xt[:], in_=x[rows, :])
                x16 = pool.tile([P, P], dt.float16, tag="x16")
                nc.vector.tensor_copy(out=x16[:], in_=xt[:])
                ps_xt = pp.tile([P, P], dt.float16, tag="tp")
                nc.tensor.transpose(out=ps_xt[:], in_=x16[:], identity=ident[:])
                xT = pool.tile([P, P], dt.float16, tag="xT")
                nc.scalar.copy(out=xT[:], in_=ps_xt[:])
                ot = pool.tile([P, 72], dt.float32, tag="ot")
                # h = xT.T @ W1   [node, f] -> fp16 into slots 0:64
                ps_h = pp.tile([P, P], dt.float32, tag="h")
                nc.tensor.matmul(out=ps_h[:], lhsT=xT[:], rhs=w116[:],
                                 start=True, stop=True)
                nc.scalar.copy(out=ot[:, 0:64].bitcast(dt.float16), in_=ps_h[:])
                # hT = W1.T @ xT  [f, node]
                ps_hT = pp.tile([P, P], dt.float32, tag="hT")
                nc.tensor.matmul(out=ps_hT[:], lhsT=w116[:], rhs=xT[:],
                                 start=True, stop=True)
                hT16 = pool.tile([P, P], dt.float16, tag="hT16")
                nc.scalar.copy(out=hT16[:], in_=ps_hT[:])
                # asadT = amat.T @ hT  [8, node]
                ps_a = pp.tile([8, P], dt.float32, tag="aT")
                nc.tensor.matmul(out=ps_a[:], lhsT=am16[:], rhs=hT16[:],
                                 start=True, stop=True)
                aT16 = pool.tile([8, P], dt.float16, tag="aT16")
                nc.scalar.copy(out=aT16[:], in_=ps_a[:])
                ps_at = pp.tile([P, 8], dt.float16, tag="a")
                nc.tensor.transpose(out=ps_at[:], in_=aT16[:],
                                    identity=ident[:8, :8])
                nc.vector.tensor_copy(out=ot[:, 64:72], in_=ps_at[:])
                nc.sync.dma_start(out=nout[rows, :], in_=ot[:])

            if reps == 1:
                node_body()
            else:
                with tc.For_i(0, reps, 1):
                    node_body()
    nc.compile()
    return nc


def build_edge_kernel(meta, layer, n, nd_pad, b_nonzero, reps=1):
    """L2/L3: one attention layer over the core's dst shard.

    layer==1: H=4 heads; epilogue -> [x2 fp16 x128 | as2 | ad2] rows (66 f32).
    layer==2: H=1; epilogue -> @W2 + bias -> out f32 [nd_pad, 128].
    """
    H = 4 if layer == 1 else 1
    RW = P + H
    ntt, nb = meta["ntt"], meta["nb"]
    tiles = meta["tiles"]
    groups = _gather_groups(meta)

    nc = bacc.Bacc("TRN2", target_bir_lowering=False, debug=False)
    table = nc.dram_tensor("table", [n, ROW_SLOTS], dt.float32,
                           kind="ExternalInput").ap()
    idx_d = nc.dram_tensor("idx16", [128, ntt * 8], dt.int16,
                           kind="ExternalInput").ap()
    dstl_d = nc.dram_tensor("dstl", [128, ntt], dt.float16,
                            kind="ExternalInput").ap()
    adrow_d = nc.dram_tensor("adrow", [nd_pad, 4], dt.float32,
                             kind="ExternalInput").ap()
    if layer == 1:
        ws2_d = nc.dram_tensor("ws2", [1, 2 * P], dt.float32,
                               kind="ExternalInput").ap()
        b1_d = nc.dram_tensor("b1", [1, P], dt.float32, kind="ExternalInput").ap()
        x2o = nc.dram_tensor("x2m", [nd_pad, 66], dt.float32,
                             kind="ExternalOutput").ap()
    else:
        w2_d = nc.dram_tensor("w2", [P, P], dt.float32, kind="ExternalInput").ap()
        b2_d = nc.dram_tensor("b2", [1, P], dt.float32, kind="ExternalInput").ap()
        outo = nc.dram_tensor("out", [nd_pad, P], dt.float32,
                              kind="ExternalOutput").ap()

    with tile.TileContext(nc) as tc:
        with tc.tile_pool(name="const", bufs=1) as cpool, \
             tc.tile_pool(name="res", bufs=1) as rpool, \
             tc.tile_pool(name="g", bufs=3) as gpool, \
             tc.tile_pool(name="w", bufs=3) as wpool, \
             tc.tile_pool(name="bl", bufs=2) as bpool, \
             tc.tile_pool(name="psum", bufs=2, space="PSUM") as pp, \
             tc.tile_pool(name="pst", bufs=4, space="PSUM") as ppt:
            from concourse.masks import make_identity
            ident = cpool.tile([P, P], dt.float16)
            make_identity(nc, ident[:])
            iota_i = cpool.tile([P, P], dt.int16)
            nc.gpsimd.iota(iota_i[:], pattern=[[1, P]], base=0,
                           channel_multiplier=0)
            iota16 = cpool.tile([P, P], dt.float16)
            nc.vector.tensor_copy(out=iota16[:], in_=iota_i[:])

            if layer == 1:
                ws2 = cpool.tile([P, 2 * P], dt.float32)
                nc.sync.dma_start(
                    out=ws2[:], in_=ws2_d[0:1, :].to_broadcast([P, 2 * P]))
                b1t = cpool.tile([P, P], dt.float32)
                nc.sync.dma_start(out=b1t[:],
                                  in_=b1_d[0:1, :].to_broadcast([P, P]))
            else:
                w2f = cpool.tile([P, P], dt.float32)
                nc.sync.dma_start(out=w2f[:], in_=w2_d[:])
                w216 = cpool.tile([P, P], dt.float16)
                nc.vector.tensor_copy(out=w216[:], in_=w2f[:])
                b2t = cpool.tile([P, P], dt.float32)
                nc.sync.dma_start(out=b2t[:], in_=b2_d[0:1, :].to_broadcast([P, P]))

            idx_sb = rpool.tile([128, ntt * 8], dt.int16)
            nc.sync.dma_start(out=idx_sb[:], in_=idx_d[:])
            dstl_sb = rpool.tile([128, ntt], dt.float16)
            nc.sync.dma_start(out=dstl_sb[:], in_=dstl_d[:])
            # all blocks' ad columns resident: [p, b, h]
            adall = rpool.tile([128, nb * H], dt.float16)
            adall_f = rpool.tile([128, nb * 4], dt.float32)
            nc.sync.dma_start(
                out=adall_f[:].rearrange("p (b h) -> p b h", h=4),
                in_=adrow_d[:].rearrange("(b p) h -> p b h", p=P))
            nc.vector.tensor_copy(
                out=adall[:].rearrange("p (b h) -> p b h", h=H),
                in_=adall_f[:].rearrange("p (b h) -> p b h", h=4)[:, :, 0:H])

            # per-block accumulator strip in SBUF
            accs = rpool.tile([128, nb * RW], dt.float32)

            lo_view = table[0:LO_ROWS, :]
            hi_view = table[LO_ROWS:n, :]

            def layer_body():
                nc.vector.memset(accs[:], 0.0)
                for (gt0, gn, sd) in groups:
                    src_view = lo_view if sd == 0 else hi_view
                    gbuf = gpool.tile([128, GMAX * ROW_SLOTS], dt.float32,
                                      tag="gb")
                    nc.gpsimd.dma_gather(
                        out_ap=gbuf[:, :gn * ROW_SLOTS].rearrange(
                            "p (n e) -> p n e", e=ROW_SLOTS),
                        in_ap=src_view,
                        idxs_ap=idx_sb[:, gt0 * 8:(gt0 + gn) * 8],
                        num_idxs=gn * P,
                        num_idxs_reg=gn * P,
                        elem_size=ROW_SLOTS,
                        single_packet=False,
                    )
                    g16 = gbuf[:, :gn * ROW_SLOTS].bitcast(dt.float16)

                    m01 = wpool.tile([128, GMAX * P], dt.float16, tag="m01")
                    nc.vector.tensor_tensor(
                        out=m01[:, :gn * P].rearrange("p (n e) -> p n e", e=P),
                        in0=iota16[:].unsqueeze(1).to_broadcast([P, gn, P]),
                        in1=dstl_sb[:, gt0:gt0 + gn].unsqueeze(2).to_broadcast(
                            [P, gn, P]),
                        op=mybir.AluOpType.is_equal)

                    adx = pp.tile([P, GMAX * H], dt.float32, tag="adx")
                    for j in range(gn):
                        blk = tiles[gt0 + j][0]
                        ps_t = ppt.tile([P, P], dt.float16, tag="tp")
                        nc.tensor.transpose(
                            out=ps_t[:], in_=m01[:, j * P:(j + 1) * P],
                            identity=ident[:])
                        m01T = wpool.tile([P, P], dt.float16, tag="m01T")
                        nc.scalar.copy(out=m01T[:], in_=ps_t[:])
                        nc.tensor.matmul(
                            out=adx[:, j * H:(j + 1) * H], lhsT=m01T[:],
                            rhs=adall[:, blk * H:(blk + 1) * H],
                            start=True, stop=True)

                    e_s = wpool.tile([128, GMAX * H], dt.float32, tag="es")
                    nc.vector.tensor_tensor(
                        out=e_s[:, :gn * H].rearrange("p (n e) -> p n e", e=H),
                        in0=gbuf[:, :gn * ROW_SLOTS].rearrange(
                            "p (n e) -> p n e", e=ROW_SLOTS)[:, :, SL_AS:SL_AS + H],
                        in1=adx[:, :gn * H].rearrange("p (n e) -> p n e", e=H),
                        op=mybir.AluOpType.add)
                    e_m = wpool.tile([128, GMAX * H], dt.float32, tag="em")
                    nc.vector.tensor_scalar_mul(
                        out=e_m[:, :gn * H], in0=e_s[:, :gn * H],
                        scalar1=NEG_SLOPE)
                    nc.vector.tensor_tensor(
                        out=e_s[:, :gn * H], in0=e_s[:, :gn * H],
                        in1=e_m[:, :gn * H], op=mybir.AluOpType.max)
                    ex16 = wpool.tile([128, GMAX * H], dt.float16, tag="ex")
                    nc.scalar.activation(ex16[:, :gn * H], e_s[:, :gn * H],
                                         mybir.ActivationFunctionType.Exp)

                    rhs = wpool.tile([128, GMAX * RW], dt.float16, tag="rhs")
                    nc.vector.tensor_tensor(
                        out=rhs[:, :gn * RW].rearrange(
                            "p (n e) -> p n e", e=RW)[:, :, 0:P].rearrange(
                            "p n (h c) -> p n h c", h=H),
                        in0=g16.rearrange("p (n e) -> p n e", e=2 * ROW_SLOTS)[
                            :, :, 0:P].rearrange("p n (h c) -> p n h c", h=H),
                        in1=ex16[:, :gn * H].rearrange(
                            "p (n h) -> p n h", h=H).unsqueeze(3).to_broadcast(
                            [128, gn, H, P // H]),
                        op=mybir.AluOpType.mult)
                    nc.vector.tensor_copy(
                        out=rhs[:, :gn * RW].rearrange(
                            "p (n e) -> p n e", e=RW)[:, :, P:RW],
                        in_=ex16[:, :gn * H].rearrange("p (n h) -> p n h", h=H))

                    # main scatter matmuls: per contiguous block piece
                    j = 0
                    while j < gn:
                        blk = tiles[gt0 + j][0]
                        k = j
                        while k < gn and tiles[gt0 + k][0] == blk:
                            k += 1
                        pacc = pp.tile([P, RW], dt.float32, tag="acc")
                        for q in range(j, k):
                            nc.tensor.matmul(
                                out=pacc[:],
                                lhsT=m01[:, q * P:(q + 1) * P],
                                rhs=rhs[:, q * RW:(q + 1) * RW],
                                start=(q == j), stop=(q == k - 1))
                        nc.vector.tensor_tensor(
                            out=accs[:, blk * RW:(blk + 1) * RW],
                            in0=accs[:, blk * RW:(blk + 1) * RW],
                            in1=pacc[:], op=mybir.AluOpType.add)
                        j = k

                # ---- epilogue over all blocks ----
                for b in range(nb):
                    acc = accs[:, b * RW:(b + 1) * RW]
                    rec = bpool.tile([P, H], dt.float32, tag="rec")
                    nc.vector.reciprocal(out=rec[:], in_=acc[:, P:RW])
                    xr = bpool.tile([P, P], dt.float32, tag="xr")
                    nc.vector.tensor_tensor(
                        out=xr[:].rearrange("p (h c) -> p h c", h=H),
                        in0=acc[:, 0:P].rearrange("p (h c) -> p h c", h=H),
                        in1=rec[:].unsqueeze(2).to_broadcast([P, H, P // H]),
                        op=mybir.AluOpType.mult)
                    rows = slice(b * P, (b + 1) * P)
                    if layer == 1:
                        if b_nonzero:
                            nc.vector.tensor_tensor(
                                out=xr[:], in0=xr[:], in1=b1t[:],
                                op=mybir.AluOpType.add)
                        nc.vector.tensor_scalar_max(out=xr[:], in0=xr[:],
                                                    scalar1=0.0)
                        om = bpool.tile([P, 66], dt.float32, tag="om")
                        nc.vector.tensor_copy(out=om[:, 0:64].bitcast(dt.float16),
                                              in_=xr[:])
                        t2 = bpool.tile([P, 2 * P], dt.float32, tag="t2")
                        nc.vector.tensor_tensor(
                            out=t2[:].rearrange("p (a c) -> p a c", a=2),
                            in0=xr[:].unsqueeze(1).to_broadcast([P, 2, P]),
                            in1=ws2[:].rearrange("p (a c) -> p a c", a=2),
                            op=mybir.AluOpType.mult)
                        nc.vector.tensor_reduce(
                            out=om[:, 64:66],
                            in_=t2[:].rearrange("p (a c) -> p a c", a=2),
                            axis=mybir.AxisListType.X, op=mybir.AluOpType.add)
                        nc.sync.dma_start(out=x2o[rows, :], in_=om[:])
                    else:
                        x16b = bpool.tile([P, P], dt.float16, tag="xab")
                        nc.vector.tensor_copy(out=x16b[:], in_=xr[:])
                        ps_t2 = ppt.tile([P, P], dt.float16, tag="tp")
                        nc.tensor.transpose(out=ps_t2[:], in_=x16b[:],
                                            identity=ident[:])
                        aggT = bpool.tile([P, P], dt.float16, tag="aggT")
                        nc.scalar.copy(out=aggT[:], in_=ps_t2[:])
                        ps_o = pp.tile([P, P], dt.float32, tag="acc")
                        nc.tensor.matmul(out=ps_o[:], lhsT=aggT[:], rhs=w216[:],
                                         start=True, stop=True)
                        ot = bpool.tile([P, P], dt.float32, tag="ot")
                        if b_nonzero:
                            nc.vector.tensor_tensor(
                                out=ot[:], in0=ps_o[:], in1=b2t[:],
                                op=mybir.AluOpType.add)
                        else:
                            nc.vector.tensor_copy(out=ot[:], in_=ps_o[:])
                        nc.sync.dma_start(out=outo[rows, :], in_=ot[:])

            if reps == 1:
                layer_body()
            else:
                with tc.For_i(0, reps, 1):
                    layer_body()
    nc.compile()
    return nc


# ----------------------------------------------------------------------------
# host orchestration
# ----------------------------------------------------------------------------

def _assemble_table(n, feat16, asv, adv):
    t = np.zeros((n, ROW_SLOTS), dtype=np.float32)
    t[:, SL_FEAT:SL_FEAT + 64] = np.ascontiguousarray(feat16).view(np.float32)
    t[:, SL_ONES:SL_ONES + 2] = F16_ONES_F32
    t[:, SL_AS:SL_AS + asv.shape[1]] = asv
    t[:, SL_AD:SL_AD + adv.shape[1]] = adv
    return t


def kernel(x, edge_index, W1, att_src1, att_dst1, b1, W2, att_src2, att_dst2, b2):
    x = np.asarray(x, np.float32)
    n = x.shape[0]
    ei = np.asarray(edge_index).astype(np.int64)
    loops = np.arange(n, dtype=np.int64)
    src = np.concatenate([ei[0], loops])
    dst = np.concatenate([ei[1], loops])
    W1 = np.asarray(W1, np.float32)
    W2 = np.asarray(W2, np.float32)
    a_s1 = np.asarray(att_src1, np.float32).reshape(4, 32)
    a_d1 = np.asarray(att_dst1, np.float32).reshape(4, 32)
    b1 = np.asarray(b1, np.float32).reshape(-1)
    b2 = np.asarray(b2, np.float32).reshape(-1)
    a_s2 = np.asarray(att_src2, np.float32).reshape(-1)
    a_d2 = np.asarray(att_dst2, np.float32).reshape(-1)

    meta = _prep_edges(src, dst, n, N_CORES)
    nd, nb = meta["nd"], meta["nb"]
    nd_pad = nb * P

    # L1: node kernel (sharded by node)
    nc1 = build_node_kernel(nd_pad)
    amat = np.zeros((P, 8), dtype=np.float32)
    for h in range(4):
        amat[h * 32:(h + 1) * 32, h] = a_s1[h]
        amat[h * 32:(h + 1) * 32, 4 + h] = a_d1[h]
    in1 = []
    for c in range(N_CORES):
        xs = np.zeros((nd_pad, P), np.float32)
        xs[:nd] = x[c * nd:(c + 1) * nd]
        in1.append({"x": xs, "w1": W1, "amat": amat})
    r1 = _execute(nc1, in1)

    nodeout = np.concatenate([r1[c]["nodeout"][:nd] for c in range(N_CORES)])
    h16 = np.ascontiguousarray(nodeout[:, 0:64]).view(np.float16)
    asad1 = nodeout[:, 64:72]
    table1 = _assemble_table(n, h16, asad1[:, 0:4], asad1[:, 4:8])

    # L2: edge layer 1
    ws2 = np.concatenate([W2 @ a_s2, W2 @ a_d2]).astype(np.float32).reshape(1, -1)
    nc2 = build_edge_kernel(meta, 1, n, nd_pad, b_nonzero=bool(np.any(b1)))
    in2 = []
    for c in range(N_CORES):
        adrow = np.zeros((nd_pad, 4), np.float32)
        adrow[:nd] = asad1[c * nd:(c + 1) * nd, 4:8]
        in2.append({"table": table1, "idx16": meta["idx16"][c],
                    "dstl": meta["dstl"][c], "adrow": adrow, "ws2": ws2,
                    "b1": b1.reshape(1, -1)})
    r2 = _execute(nc2, in2)

    x2m = np.concatenate([r2[c]["x2m"][:nd] for c in range(N_CORES)])
    x2_16 = np.ascontiguousarray(x2m[:, 0:64]).view(np.float16)
    asad2 = x2m[:, 64:66]
    table2 = _assemble_table(n, x2_16, asad2[:, 0:1], asad2[:, 1:2])

    # L3: edge layer 2
    nc3 = build_edge_kernel(meta, 2, n, nd_pad, b_nonzero=bool(np.any(b2)))
    in3 = []
    for c in range(N_CORES):
        adrow = np.zeros((nd_pad, 4), np.float32)
        adrow[:nd, 0] = asad2[c * nd:(c + 1) * nd, 1]
        in3.append({"table": table2, "idx16": meta["idx16"][c],
                    "dstl": meta["dstl"][c], "adrow": adrow,
                    "w2": W2, "b2": b2.reshape(1, -1)})
    r3 = _execute(nc3, in3)

    out = np.concatenate([r3[c]["out"][:nd] for c in range(N_CORES)])
    return out.astype(np.float32)
